# revision 91
# baseline (speedup 1.0000x reference)
"""Trainium2 Bass kernel: 3x3 valid conv (N=32, Cin=64, 128x128 -> Cout=128,
126x126) + bias, *0.5, then min over output channels.

Final strategy (data-parallel over batch, 4 images per core on 8 cores):
- Conv in fp8e4m3 with DoubleRow matmuls: x is packed host-side as ONE fp8
  plane per image ([x | x shift-W] on partitions 0-63/64-127). The DoubleRow
  rhs is an overlapping strided AP (K-half dim stride = W), so half-0 reads
  the plane at column c and half-1 at c+W: one K=256 DoubleRow matmul per kw
  covers taps (0,kw),(1,kw),(2,kw) (the half-1 lower slot carries zero
  weights). The whole 3x3x64 conv is 3 matmuls per 512-position chunk --
  ~3x fewer PE cycles than bf16 -- and x DMA is 2.1 MB/image (4x less than
  the bf16 two-tile scheme). e4m3 end-to-end rel err is 0.0155 (< 2e-2
  gate; verified on CPU and HW).
- Weight-major rounds of G=4 chunks amortize LDWEIGHTS (this toolchain
  compiles with --enable-ldw-opt=false). SCALE=0.5 is folded into the
  ScalarE activation (scale=0.5, bias pre-scaled) to keep the fp8 weights
  in e4m3's normal range.
- ScalarE drains PSUM->SBUF bf16 with bias; the min over cout (partitions)
  is split across two balanced paths: half the rounds use per-chunk PE
  transposes + a small DVE reduce (collector `coll`, baseline-style), half
  use DVE StreamTranspose (32x32 blocks) + strided reduce into
  M[32i+a, F] = blockwise min of flat position 32F+a, finalized by 4 small
  PE transposes + one strided 4D DVE reduce per image. Outputs land in DRAM
  in permuted layouts the host unscrambles with cheap reshapes.
"""

import numpy as np
import ml_dtypes

N_CORES = 8
IMGS = 4  # images per core
H = W = 128
CIN = 64
COUT = 128
HW = H * W
XCOLS = HW + 512  # zero padding so the last chunk's shifted reads stay in-bounds
OUT = 126
NCHUNK = 32  # chunks of 512 flat output positions per image
G = 4  # chunks per weight-major round
PE_ROUNDS = (1, 3, 5, 7)  # rounds whose min-reduce runs on the PE-transpose path

_bf16 = ml_dtypes.bfloat16
_CACHE: dict = {}


def _build_module(
    imgs=IMGS,
    repeats=1,
    dma_split=8,
    conv_only=False,
    dma_only=False,
    one_tile=False,
    fp8=True,
    fp8_planes=1,
    pe_rounds=PE_ROUNDS,
):
    # NOTE: EngineType.Pool on TRN2 is the GPSIMD slot — ISA vector ops like
    # TensorTensor do not codegen there, so vector epilogue work is DVE-only.
    # The min-over-cout epilogue is split between two paths to balance PE vs
    # DVE: rounds in pe_rounds use per-chunk PE transposes + a small DVE
    # reduce (baseline-style, collector `coll`), the rest use DVE
    # StreamTranspose + strided reduce (collector `M`).
    round_plan = [(c0, 4) for c0 in range(0, NCHUNK, 4)]
    import concourse.bass as bass
    import concourse.mybir as mybir
    import concourse.tile as tile
    from concourse import bacc
    from concourse.masks import make_identity

    f32 = mybir.dt.float32
    bf16 = mybir.dt.bfloat16
    f8 = mybir.dt.float8e4

    nc = bacc.Bacc("TRN2", target_bir_lowering=False, debug=False)
    if fp8:
        # fp8 planes: plane0 = [x | x@W], plane1 = [x@W | x@2W]; one
        # DoubleRow matmul per kw covers taps (0,kw),(1,kw),(2,kw).
        # With fp8_planes=1 only plane0 is materialized and the DoubleRow
        # rhs is an overlapping strided AP (plane stride = W), halving DMA.
        if fp8_planes == 1:
            xf_d = nc.dram_tensor("xf1", [imgs, 128, XCOLS], f8, kind="ExternalInput")
        else:
            xf_d = nc.dram_tensor(
                "xf", [imgs, 128, 2, XCOLS], f8, kind="ExternalInput"
            )
        wf_d = nc.dram_tensor("wf", [128, 2, 3 * 128], f8, kind="ExternalInput")
    x_d = (
        None
        if fp8
        else nc.dram_tensor("x", [imgs, 128, XCOLS], bf16, kind="ExternalInput")
    )
    x2_d = (
        None
        if (one_tile or fp8)
        else nc.dram_tensor("x2", [imgs, 128, XCOLS], bf16, kind="ExternalInput")
    )
    w_d = nc.dram_tensor("w", [128, 8 * 128], bf16, kind="ExternalInput")
    b_d = nc.dram_tensor("b", [128, 1], f32, kind="ExternalInput")
    o_d = nc.dram_tensor("out", [imgs, 4, 128, 32], f32, kind="ExternalOutput")
    oc_d = nc.dram_tensor("out_c", [imgs, 128, 128], f32, kind="ExternalOutput")

    with tile.TileContext(nc) as tc:
        with (
            tc.tile_pool(name="xp", bufs=3 if (fp8 and fp8_planes == 1) else 2) as xp,
            tc.tile_pool(name="wp", bufs=1) as wp,
            tc.tile_pool(name="cp", bufs=3) as cp,
            tc.tile_pool(name="tp", bufs=3) as tp,
            tc.tile_pool(name="mp", bufs=2) as mp,
            tc.tile_pool(name="fp", bufs=2) as fp,
            tc.tile_pool(name="psA", bufs=4, space=bass.MemorySpace.PSUM) as psA,
            tc.tile_pool(name="psB", bufs=2, space=bass.MemorySpace.PSUM) as psB,
            tc.tile_pool(name="psM", bufs=1, space=bass.MemorySpace.PSUM) as psM,
            tc.tile_pool(name="psC", bufs=1, space=bass.MemorySpace.PSUM) as psC,
        ):
            wt = wp.tile([128, 8 * 128], bf16)
            nc.sync.dma_start(wt[:], w_d[:])
            if fp8:
                wf = wp.tile([128, 2, 3 * 128], f8)
                nc.sync.dma_start(wf[:], wf_d[:])
            bt = wp.tile([128, 1], f32)
            nc.sync.dma_start(bt[:], b_d[:])
            idb = wp.tile([128, 128], bf16)
            make_identity(nc, idb[:])
            idf = wp.tile([128, 128], f32)
            make_identity(nc, idf[:])

            import contextlib

            rep_ctx = (
                tc.For_i(0, repeats) if repeats > 1 else contextlib.nullcontext()
            )
            with rep_ctx:
              def finalize_q(MT, fin, M, n, q):
                # MT[p, q, f] = M[f, 128q + p]; then min over the 4
                # cout-blocks i (f = 32i + a) -> fin[p, q, a] which is the
                # result for oh = 32q + p//4, ow = 32*(p%4) + a
                nc.tensor.transpose(
                    MT[:, q], M[:, q * 128 : (q + 1) * 128], idb[:]
                )
                nc.vector.tensor_reduce(
                    fin[:, q],
                    MT[:, q].rearrange("p (i a) -> p a i", a=32),
                    axis=mybir.AxisListType.X,
                    op=mybir.AluOpType.min,
                )
                nc.sync.dma_start(o_d[n, q], fin[:, q])

              # last image: stage each quarter of M as soon as its rounds
              # land (F progresses 64/round at G=4; one round of slack);
              # earlier images: whole finalize deferred into the next image
              fin_stage = {2: 0, 4: 1, 6: 2}
              pending_fin = None
              pending_cfin = None

              def finalize_coll(coll, n):
                # coll is [ow, oh]; transpose to [oh, ow], write whole tile
                cf = psC.tile([128, 128], f32, tag="cf", name=f"cf_{n}")
                nc.tensor.transpose(cf[:], coll[:], idf[:])
                ob = fp.tile([128, 128], f32, tag="ob", name=f"ob_{n}")
                nc.any.tensor_copy(ob[:], cf[:])
                nc.sync.dma_start(oc_d[n], ob[:])

              for n in range(imgs):
                step = -(-XCOLS // dma_split)
                if fp8 and fp8_planes == 1:
                    xft = xp.tile([128, XCOLS], f8, tag="xf", name=f"xf_{n}")
                    for s in range(dma_split):
                        c0, c1 = s * step, min((s + 1) * step, XCOLS)
                        nc.sync.dma_start(xft[:, c0:c1], xf_d[n, :, c0:c1])
                    xt = xt2 = None
                elif fp8:
                    xft = xp.tile([128, 2, XCOLS], f8, tag="xf", name=f"xf_{n}")
                    for s in range(dma_split):
                        c0, c1 = s * step, min((s + 1) * step, XCOLS)
                        nc.sync.dma_start(
                            xft[:, :, c0:c1], xf_d[n, :, :, c0:c1]
                        )
                    xt = xt2 = None
                else:
                    xt = xp.tile([128, XCOLS], bf16)
                    xt2 = (
                        None
                        if one_tile
                        else xp.tile([128, XCOLS], bf16, tag="xt2", name=f"xt2_{n}")
                    )
                    for s in range(dma_split):
                        c0, c1 = s * step, min((s + 1) * step, XCOLS)
                        nc.sync.dma_start(xt[:, c0:c1], x_d[n, :, c0:c1])
                        if xt2 is None:
                            pass
                        elif dma_only == 2:
                            nc.sync.dma_start(xt2[:, c0:c1], xt[:, c0:c1])
                        else:
                            nc.sync.dma_start(xt2[:, c0:c1], x2_d[n, :, c0:c1])
                n_dve_rounds = len(round_plan) - len(pe_rounds)
                M = MT = fin = coll = None
                if n_dve_rounds > 0 and not (conv_only or dma_only):
                    M = mp.tile([128, 512], bf16, tag="M", name=f"M_{n}")
                    MT = psM.tile([128, 4, 128], bf16, tag="mt", name=f"mt_{n}")
                    fin = fp.tile([128, 4, 32], f32, tag="fin", name=f"fin_{n}")
                if pe_rounds and not (conv_only or dma_only):
                    coll = mp.tile([128, 128], f32, tag="coll", name=f"coll_{n}")

                if dma_only:
                    if n == imgs - 1:
                        dr2 = cp.tile([128, 32], f32, tag="dr2", name=f"dr2_{n}")
                        nc.any.tensor_copy(dr2[:], xt[:, 0:32])
                        nc.sync.dma_start(o_d[0, 0, :, :], dr2[:])
                    continue
                for r, (c0, cg) in enumerate(round_plan):
                    pss = [
                        psA.tile([128, 512], f32, tag="ps", name=f"ps_{n}_{c0}_{g}")
                        for g in range(cg)
                    ]
                    # weight-major: each stationary weight slot loads once
                    # per round and streams cg moving chunks
                    if fp8:
                        import bass_rust as _br

                        # snake the kw order on alternating rounds: the last
                        # stationary of round r equals the first of round
                        # r+1, so the framework's ldweights dedup saves a
                        # reload at every clean round boundary
                        kws = (0, 1, 2) if r % 2 == 0 else (2, 1, 0)
                        for i, kw in enumerate(kws):
                            for g in range(cg):
                                base = (c0 + g) * 512
                                if fp8_planes == 1:
                                    v = xft[:, base + kw : base + kw + 512]
                                    rhs = _br.AP(
                                        tensor=v.tensor,
                                        offset=v.offset,
                                        ap=[
                                            [v.ap[0][0], 128],
                                            [W, 2],
                                            [1, 512],
                                        ],
                                    )
                                else:
                                    rhs = xft[:, 0:2, base + kw : base + kw + 512]
                                nc.tensor.matmul(
                                    pss[g][:],
                                    wf[:, 0:2, kw * 128 : (kw + 1) * 128],
                                    rhs,
                                    start=(i == 0),
                                    stop=(i == 2),
                                    perf_mode=mybir.MatmulPerfMode.DoubleRow,
                                )
                    else:
                        for kw in range(3):
                            for g in range(cg):
                                base = (c0 + g) * 512
                                nc.tensor.matmul(
                                    pss[g][:],
                                    wt[:, kw * 128 : (kw + 1) * 128],
                                    xt[:, base + kw : base + kw + 512],
                                    start=(kw == 0),
                                    stop=False,
                                )
                    if fp8:
                        pass
                    elif one_tile:
                        # row-2 taps via the packed upper half (x shift-W) at
                        # offsets W+kw; lower-half weights are zero
                        for kw in range(3):
                            for g in range(cg):
                                base = (c0 + g) * 512
                                nc.tensor.matmul(
                                    pss[g][:],
                                    wt[:, (5 + kw) * 128 : (6 + kw) * 128],
                                    xt[:, base + W + kw : base + W + kw + 512],
                                    start=False,
                                    stop=(kw == 2),
                                )
                    else:
                        for g in range(cg):
                            base = (c0 + g) * 512
                            nc.tensor.matmul(
                                pss[g][:],
                                wt[:, 3 * 128 : 4 * 128],
                                xt2[:, base + 2 * W : base + 2 * W + 512],
                                start=False,
                                stop=False,
                            )
                        for g in range(cg):
                            base = (c0 + g) * 512
                            nc.tensor.matmul(
                                pss[g][:],
                                wt[0:64, 4 * 128 : 5 * 128],
                                xt[0:64, base + 2 * W + 2 : base + 2 * W + 2 + 512],
                                start=False,
                                stop=True,
                            )
                    if conv_only:
                        # just drain each PSUM bank with a minimal ACT copy
                        if r == len(round_plan) - 1 and n == imgs - 1:
                            dr = cp.tile([128, G, 512], f32, tag="ct", name="drain")
                            for g in range(cg):
                                nc.scalar.activation(
                                    dr[:, g], pss[g][:],
                                    mybir.ActivationFunctionType.Identity,
                                )
                            nc.sync.dma_start(o_d[0, 0, :, :], dr[:, 0, 0:32])
                        continue
                    # ScalarE: PSUM -> SBUF with bias (per-partition = cout)
                    ct = cp.tile([128, G, 512], bf16, tag="ct", name=f"ct_{n}_{r}")
                    for g in range(cg):
                        nc.scalar.activation(
                            ct[:, g],
                            pss[g][:],
                            mybir.ActivationFunctionType.Identity,
                            bias=bt[:],
                            scale=0.5 if fp8 else 1.0,
                        )
                    if r in pe_rounds:
                        # PE path: per-chunk transposes to PSUM + small DVE
                        # reduce over cout into the coll collector
                        for g in range(cg):
                            c = c0 + g
                            tpp = psB.tile(
                                [128, 4, 128], bf16, tag="tp", name=f"tp_{n}_{c}"
                            )
                            for q in range(4):
                                nc.tensor.transpose(
                                    tpp[:, q],
                                    ct[:, g, q * 128 : (q + 1) * 128],
                                    idb[:],
                                )
                            nc.vector.tensor_reduce(
                                coll[:, c * 4 : c * 4 + 4],
                                tpp[:],
                                axis=mybir.AxisListType.X,
                                op=mybir.AluOpType.min,
                            )
                    else:
                        # DVE path: 32x32 block transpose + min over the 32
                        # cout of each block -> M[32i+a, F] = partial min of
                        # pos 32F+a
                        tt = tp.tile(
                            [128, G, 512], bf16, tag="tt", name=f"tt_{n}_{r}"
                        )
                        nc.vector.transpose(
                            tt[:, 0:cg].rearrange("p g s -> p (g s)"),
                            ct[:, 0:cg].rearrange("p g s -> p (g s)"),
                        )
                        nc.vector.tensor_reduce(
                            M[:, c0 * 16 : (c0 + cg) * 16],
                            tt[:, 0:cg].rearrange("p g (J a) -> p (g J) a", a=32),
                            axis=mybir.AxisListType.X,
                            op=mybir.AluOpType.min,
                        )
                    if n == imgs - 1 and r in fin_stage and M is not None:
                        finalize_q(MT, fin, M, n, fin_stage[r])
                    if n > 0 and r == 1:
                        if pending_fin is not None:
                            pM, pMT, pfin, pn = pending_fin
                            for q in range(4):
                                finalize_q(pMT, pfin, pM, pn, q)
                            pending_fin = None
                        if pending_cfin is not None:
                            finalize_coll(*pending_cfin)
                            pending_cfin = None
                if conv_only:
                    continue
                if n == imgs - 1:
                    if M is not None:
                        finalize_q(MT, fin, M, n, 3)
                    if coll is not None:
                        finalize_coll(coll, n)
                else:
                    if M is not None:
                        pending_fin = (M, MT, fin, n)
                    if coll is not None:
                        pending_cfin = (coll, n)
    nc.compile()
    return nc


def _get_nc():
    if "nc" not in _CACHE:
        _CACHE["nc"] = _build_module()
    return _CACHE["nc"]


def _pack_inputs(x, weight, bias):
    x = np.asarray(x, np.float32)
    weight = np.asarray(weight, np.float32)
    bias = np.asarray(bias, np.float32)
    n_total = x.shape[0]

    xbf = x.astype(_bf16).reshape(n_total, CIN, HW)
    xb = np.zeros((n_total, 128, XCOLS), _bf16)
    xb[:, 0:CIN, :HW] = xbf
    xb[:, CIN:128, : HW - W] = xbf[:, :, W:]  # shifted by one image row
    xb2 = np.zeros((n_total, 128, XCOLS), _bf16)
    xb2[:, 0:CIN, :HW] = xbf
    xb2[:, CIN:128, : HW - 1] = xbf[:, :, 1:]  # shifted by one element

    w_bf = (0.5 * weight).astype(_bf16)  # fold SCALE
    wpack = np.zeros((128, 8 * 128), _bf16)
    for kw in range(3):
        wpack[0:64, kw * 128 : (kw + 1) * 128] = w_bf[:, :, 0, kw].T
        wpack[64:128, kw * 128 : (kw + 1) * 128] = w_bf[:, :, 1, kw].T
    wpack[0:64, 3 * 128 : 4 * 128] = w_bf[:, :, 2, 0].T  # (2,0) lower
    wpack[64:128, 3 * 128 : 4 * 128] = w_bf[:, :, 2, 1].T  # (2,1) upper (shift-1 tile)
    wpack[0:64, 4 * 128 : 5 * 128] = w_bf[:, :, 2, 2].T  # (2,2) single
    for kw in range(3):  # one-tile mode: row-2 taps on the upper half only
        wpack[64:128, (5 + kw) * 128 : (6 + kw) * 128] = w_bf[:, :, 2, kw].T
    bias_f = (0.5 * bias).astype(np.float32).reshape(128, 1)

    # fp8 packing: x quantized from fp32; weights UNSCALED (the 0.5 folds
    # into the ScalarE activation) to stay in e4m3's normal range.
    _f8 = ml_dtypes.float8_e4m3
    xq = x.astype(_f8).reshape(n_total, CIN, HW)
    xf = np.zeros((n_total, 128, 2, XCOLS), _f8)
    xf[:, 0:CIN, 0, :HW] = xq
    xf[:, CIN:128, 0, : HW - W] = xq[:, :, W:]  # x shift-W
    xf[:, 0:CIN, 1, : HW - W] = xq[:, :, W:]  # x shift-W (zero-weight slot)
    xf[:, CIN:128, 1, : HW - 2 * W] = xq[:, :, 2 * W :]  # x shift-2W
    w_f8 = weight.astype(_f8)
    wfpack = np.zeros((128, 2, 3 * 128), _f8)
    for kw in range(3):
        wfpack[0:64, 0, kw * 128 : (kw + 1) * 128] = w_f8[:, :, 0, kw].T
        wfpack[64:128, 0, kw * 128 : (kw + 1) * 128] = w_f8[:, :, 1, kw].T
        wfpack[64:128, 1, kw * 128 : (kw + 1) * 128] = w_f8[:, :, 2, kw].T

    in_maps = []
    for core in range(N_CORES):
        in_maps.append(
            {
                "x": np.ascontiguousarray(xb[core * IMGS : (core + 1) * IMGS]),
                "x2": np.ascontiguousarray(xb2[core * IMGS : (core + 1) * IMGS]),
                "xf": np.ascontiguousarray(xf[core * IMGS : (core + 1) * IMGS]),
                "xf1": np.ascontiguousarray(
                    xf[core * IMGS : (core + 1) * IMGS, :, 0]
                ),
                "w": wpack,
                "wf": wfpack,
                "b": bias_f,
            }
        )
    return in_maps


def _unpack_out(o_m, o_c, pe_rounds=PE_ROUNDS):
    """o_m [n, 4, 128, 32]: value for oh = 32q + p//4, ow = 32*(p%4)+a at
    [n, q, p, a]; a straight reshape gives [oh, ow]. o_c [n, 128, 128] is
    already [oh, ow] and holds the rows of the PE-path rounds."""
    n = o_m.shape[0]
    out = o_m.reshape(n, 128, 128).copy()
    for r in pe_rounds:
        out[:, 16 * r : 16 * r + 16] = o_c[:, 16 * r : 16 * r + 16]
    return out[:, :OUT, :OUT]


def _module_io(nc):
    import concourse.mybir as mybir

    part_name = nc.partition_id_tensor.name if nc.partition_id_tensor else None
    in_names, out_names, out_avals = [], [], []
    for alloc in nc.m.functions[0].allocations:
        if not isinstance(alloc, mybir.MemoryLocationSet):
            continue
        name = alloc.memorylocations[0].name
        if alloc.kind == "ExternalInput":
            if name != part_name:
                in_names.append(name)
        elif alloc.kind == "ExternalOutput":
            out_names.append(name)
            out_avals.append((tuple(alloc.tensor_shape), mybir.dt.np(alloc.dtype)))
    return in_names, out_names, out_avals, part_name


def _build_runner(nc):
    """Per-device jitted runners (no shard_map/mesh: pure data parallel)."""
    import jax
    from concourse.bass2jax import (
        _bass_exec_p,
        install_neuronx_cc_hook,
        partition_id_tensor,
    )

    install_neuronx_cc_hook()
    in_names, out_names, out_avals, part_name = _module_io(nc)
    all_names = tuple(in_names) + tuple(out_names)
    if part_name is not None:
        all_names = all_names + (part_name,)
    avals = tuple(
        jax.core.ShapedArray(shape, dtype) for shape, dtype in out_avals
    )

    def body(*args):
        extra = [partition_id_tensor()] if part_name else []
        outs = _bass_exec_p.bind(
            *args,
            *extra,
            out_avals=avals,
            in_names=all_names,
            out_names=tuple(out_names),
            lowering_input_output_aliases=(),
            sim_require_finite=True,
            sim_require_nnan=True,
            nc=nc,
        )
        return tuple(outs)

    fns = [jax.jit(body, device=d) for d in jax.devices()[:N_CORES]]
    return fns, in_names, out_names, out_avals


def _run_per_device(nc, in_maps):
    if "runner" not in _CACHE:
        _CACHE["runner"] = _build_runner(nc)
    fns, in_names, out_names, out_avals = _CACHE["runner"]
    zeros = [np.zeros(shape, dtype) for shape, dtype in out_avals]
    futs = []
    for core in range(N_CORES):
        args = [np.asarray(in_maps[core][n]) for n in in_names] + zeros
        futs.append(fns[core](*args))
    return [
        {name: np.asarray(f[i]) for i, name in enumerate(out_names)} for f in futs
    ]


def _run(x, weight, bias, trace=False):
    nc = _get_nc()
    in_maps = _pack_inputs(x, weight, bias)
    results = None
    last_exc = None
    for _attempt in range(3):
        try:
            results = _run_per_device(nc, in_maps)
            break
        except Exception as e:  # transient device errors: retry
            last_exc = e
    if results is None:
        # fall back to the spmd/shard_map path
        from concourse.bass_utils import run_bass_kernel_spmd

        try:
            res = run_bass_kernel_spmd(
                nc, in_maps, core_ids=list(range(N_CORES)), trace=False
            )
            results = res.results
        except Exception:
            raise last_exc
    out = np.empty((N_CORES * IMGS, 1, OUT, OUT), np.float32)
    for core in range(N_CORES):
        out[core * IMGS : (core + 1) * IMGS, 0] = _unpack_out(
            results[core]["out"], results[core]["out_c"]
        )
    return out, None


def kernel(x, weight, bias):
    out, _ = _run(x, weight, bias, trace=False)
    return out
